# revision 67
# baseline (speedup 1.0000x reference)
"""GraphWaveNet kernel for Trainium2 (Bass/Tile), 8 NeuronCores.

Design: edge sharding by SOURCE block + per-layer ReduceScatter (no
AllGathers, no replicated compute).

- Only t=11 survives the final 1x1 conv and the GCN doesn't mix time, so
  the conv stack is evaluated at t in {10,11} only and the GCN runs on
  B=4 slices packed as D=256 columns (4 slices x 64 feats).
- GCN identity: with Hs = dsq*h, agg_n = dsq_n * (sum_{e->n} Hs[src_e]
  + Hs[n]), then @W + b + relu.
- Sharding: core k owns node rows [1280k, 1280(k+1)). The conv builds
  only the local shard of the Hs table (bf16, HBM). Edges live on the
  core owning their SRC, so gathers (dma_gather, 1024 indices/instr)
  read only the local table. One-hot P matmuls scatter 128-edge chunks
  into per-dst-block PSUM accumulators (5-block groups, one staging
  copy + one DMA per group), forming a bf16 partial aggregate over all
  10240 rows. A single bf16 ReduceScatter per layer sums the partials
  across cores and returns each core its own 1280 rows; self-loop,
  dsq scaling, weight matmul, bias and relu are then local (block-pair
  batched). Final 1x1 conv + host assembly of y.
- SPMD uniformity: the slot schedule uses S_b = max over cores of
  ceil(edges/128) per dst block; up to three blocks' overflow edges
  share one slot at static partition sub-ranges (the sub-range base is
  host-side data -- the program only sees P columns and chain flags).
  PSUM allows only one open matmul accumulation chain per 2KB bank, so
  chains are emitted in two waves whose open chains never share a bank.
- dma_gather ucode contract (queue 0): flat index j of an instruction
  lands at out[j%128, j//128] and is read from idx tile position
  [16 + j%16, j//16] (int16).
"""

import sys

sys.path.insert(0, "/opt/trn_rl_repo")

import numpy as np
import ml_dtypes

B, T, N, FIN, H, E = 4, 12, 10000, 2, 64, 80000
NCORES = 8
NB80 = 80                 # dst blocks of 128 nodes
NP = NB80 * 128           # padded node count (10240)
NSH = NP // NCORES        # node rows per core (1280)
NBC = NB80 // NCORES      # node blocks per core (10)
D = 4 * H                 # 256 = 4 slices x 64 feats
SPG = 8                   # slots (of 128 edges) per dma_gather (1024 idxs)
NIG = SPG * 128           # indices per gather instruction
ICPG = NIG // 16          # idx tile columns per gather instruction (64)

_cache = {}


def _host_prep(x, edge_index, w1, b1, w2, b2, gw1, gb1, gw2, gb2, wo, bo):
    x = np.asarray(x, np.float32)
    src = np.asarray(edge_index[0]).astype(np.int64)
    dst = np.asarray(edge_index[1]).astype(np.int64)

    deg = np.bincount(dst, minlength=N).astype(np.float64) + 1.0
    dsq = (deg ** -0.5).astype(np.float32)
    dsq_pad = np.ones(NP, dtype=np.float32)
    dsq_pad[:N] = dsq

    # ---- per-core edge partition by src owner, dst-sorted
    owner = src // NSH
    es_k, ed_k, cnt = [], [], np.zeros((NCORES, NB80), np.int64)
    for k in range(NCORES):
        m = owner == k
        es, ed = src[m], dst[m]
        o = np.argsort(ed, kind="stable")
        es, ed = es[o], ed[o]
        es_k.append(es)
        ed_k.append(ed)
        cnt[k] = np.bincount(ed // 128, minlength=NB80)

    S_b = np.maximum(1, (cnt + 127) // 128).max(axis=0)   # slots per block
    ovf_max = np.maximum(0, cnt - 128).max(axis=0)
    assert int(S_b.max()) <= 2 and int(ovf_max.max()) <= 64, \
        "overflow pairing assumes <=192 edges per (core, block)"
    # slots: tuple of mm-specs (block, first, last, pbase). A block's <=128
    # main edges fill its own slot; two blocks' overflow edges (<=64 each)
    # share one slot at partition bases 0 and 64. Groups of 5 blocks (one
    # shard-half) accumulate in one PSUM tile; group's last mm flagged by
    # gdone for the agg copy/DMA.
    # PSUM allows only one OPEN accumulation chain per 2KB bank; regions
    # w={0,1},{2,3},{4} share banks, so emit in two waves whose open chains
    # never share a bank, overflow-pairing within each wave (cross-bank).
    # overflow sub-slot layouts by count: (pbase, cap) pairs; caps must
    # cover max overflow across cores (asserted below)
    OVL = {1: ((0, 128),), 2: ((0, 64), (64, 64)),
           3: ((0, 43), (43, 43), (86, 42))}

    def group_slots(blocks):
        out = []
        for wave in ((blocks[0], blocks[2], blocks[4]),
                     (blocks[1], blocks[3])):
            for b in wave:
                out.append(((b, True, int(S_b[b]) == 1, 0, 128),))
            ovf = [b for b in wave if int(S_b[b]) == 2]
            if ovf:
                lay = OVL[len(ovf)]
                for b, (pb, cap) in zip(ovf, lay):
                    assert int(ovf_max[b]) <= cap, (b, ovf_max[b], cap)
                out.append(tuple((b, False, True, pb, cap)
                                 for b, (pb, cap) in zip(ovf, lay)))
        return out
    slots = []
    for hf in range(2):
        if hf == 1:
            while len(slots) % SPG:
                slots.append(())                # pad to gather-group boundary
            SPL = len(slots)
        for shd in range(NCORES):
            blocks = [10 * shd + 5 * hf + w for w in range(5)]
            slots.extend(group_slots(blocks))
    TOT = len(slots)
    NGI = (TOT + SPG - 1) // SPG

    # eidx layout (dma_gather ucode contract, queue 0): within gather
    # instruction gi, flat index j in [0, NIG) lives at SBUF position
    # [16 + (j % 16), gi * ICPG + j // 16]; edge j lands at out[j%128, j//128].
    NMM = sum(len(s) for s in slots)
    eidx_all = np.zeros((NCORES, 128, NGI * ICPG), np.int16)
    P_all = np.zeros((NCORES, 128, NMM * 128), np.float32)
    for k in range(NCORES):
        es, ed = es_k[k], ed_k[k]
        bounds = np.searchsorted(ed, np.arange(NB80 + 1) * 128)
        mmi = 0
        for si, specs in enumerate(slots):
            for (b, first, last, pbase, cap) in specs:
                e0, e1 = int(bounds[b]), int(bounds[b + 1])
                a = e0 if first else e0 + 128
                bb = min(a + cap, e1)
                ne = max(0, bb - a)
                if ne:
                    gi, c = divmod(si, SPG)
                    jj = c * 128 + pbase + np.arange(ne)   # flat idx in instr
                    eidx_all[k, 16 + (jj % 16), gi * ICPG + jj // 16] = \
                        (es[a:bb] - k * NSH).astype(np.int16)
                    P_all[k, pbase + np.arange(ne),
                          mmi * 128 + (ed[a:bb] - b * 128)] = 1.0
                mmi += 1
    P_all = P_all.astype(ml_dtypes.bfloat16)

    # ---- conv input: per block 8 rows (t,c) for t in {9,10,11} + 2 zero rows,
    # cols = 4 slices x 128 nodes
    xpad = np.zeros((B, 3, FIN, NP), np.float32)
    xpad[:, :, :, :N] = x[:, 9:12, :, :].transpose(0, 1, 3, 2)  # [s, ti, c, n]
    xv = xpad.reshape(B, 6, NCORES, NBC, 128)                   # [s, row, k, blk, p]
    xt_all = np.zeros((NCORES, 8, NBC * 4 * 128), np.float32)
    xt_all[:, :6] = xv.transpose(2, 1, 3, 0, 4).reshape(NCORES, 6, NBC * 4 * 128)
    xt_all = xt_all.astype(ml_dtypes.bfloat16)

    dsqk_all = dsq_pad.reshape(NCORES, NBC, 128).transpose(0, 2, 1).copy()

    # ---- weights
    W1m = np.zeros((6, 64), np.float32)
    for kk in range(3):
        for c in range(FIN):
            W1m[2 * kk + c, :] = w1[:, c, 0, kk]
    W1ab = np.zeros((8, 128), np.float32)
    W1ab[0:6, 0:64] = W1m          # A: t10 (taps t9,t10,t11)
    W1ab[2:8, 64:128] = W1m        # B: t11 (taps t10,t11,t12=pad)
    W1ab = W1ab.astype(ml_dtypes.bfloat16)

    W2m = np.zeros((128, 64), np.float32)
    W2m[:64, :] = w2[:, :, 0, 0].T
    W2m[64:, :] = w2[:, :, 0, 1].T
    W2m = W2m.astype(ml_dtypes.bfloat16)

    b1s = np.concatenate([b1, b1]).reshape(128, 1).astype(np.float32)
    b2c = np.asarray(b2, np.float32).reshape(64, 1)
    gb1s = np.concatenate([gb1, gb1]).reshape(128, 1).astype(np.float32)
    gb2s = np.concatenate([gb2, gb2]).reshape(128, 1).astype(np.float32)
    gwd1 = np.zeros((128, 128), np.float32)
    gwd1[0:64, 0:64] = gw1
    gwd1[64:128, 64:128] = gw1
    gwd1 = gwd1.astype(ml_dtypes.bfloat16)
    gwd2 = np.zeros((128, 128), np.float32)
    gwd2[0:64, 0:64] = gw2
    gwd2[64:128, 64:128] = gw2
    gwd2 = gwd2.astype(ml_dtypes.bfloat16)
    wov = np.asarray(wo, np.float32)[0, :, 0, 0]
    wod2 = np.zeros((128, 2), np.float32)
    wod2[0:64, 0] = wov
    wod2[64:128, 1] = wov
    wod2 = wod2.astype(ml_dtypes.bfloat16)

    shared = {
        "W1ab": W1ab, "W2m": W2m, "b1s": b1s, "b2c": b2c,
        "gb1s": gb1s, "gb2s": gb2s, "gwd1": gwd1, "gwd2": gwd2, "wod2": wod2,
    }
    in_maps = []
    for k in range(NCORES):
        m = dict(shared)
        m["xt"] = xt_all[k]
        m["eidx"] = eidx_all[k]
        m["P"] = P_all[k]
        m["dsqk"] = dsqk_all[k]
        in_maps.append(m)
    return in_maps, (slots, SPL), NGI, float(np.asarray(bo).reshape(-1)[0])


def _build(slots_spl, NGI, bo_f):
    slots, SPL = slots_spl
    from concourse import bass, bacc, tile
    from concourse.masks import make_identity
    import mybir

    f32, bf16, i16 = mybir.dt.float32, mybir.dt.bfloat16, mybir.dt.int16
    f32r = mybir.dt.float32r
    TOT = len(slots)
    NMM = sum(len(s) for s in slots)
    COLS = NGI * ICPG

    nc = bacc.Bacc("TRN2", target_bir_lowering=False, debug=False, num_devices=8,
                   dynamic_dma_scratch_size=65536)

    ext = {}
    for name, shape, dt in [
        ("xt", [8, NBC * 512], bf16), ("W1ab", [8, 128], bf16),
        ("W2m", [128, 64], bf16), ("b1s", [128, 1], f32), ("b2c", [64, 1], f32),
        ("gb1s", [128, 1], f32), ("gb2s", [128, 1], f32),
        ("gwd1", [128, 128], bf16), ("gwd2", [128, 128], bf16),
        ("wod2", [128, 2], bf16), ("dsqk", [128, NBC], f32),
        ("eidx", [128, COLS], i16), ("P", [128, NMM * 128], bf16),
    ]:
        ext[name] = nc.dram_tensor(name, shape, dt, kind="ExternalInput").ap()
    y_ext = nc.dram_tensor("y", [128, 4 * NBC], f32, kind="ExternalOutput").ap()
    table0 = nc.dram_tensor("table0", [NSH, D], bf16).ap()
    table1 = nc.dram_tensor("table1", [NSH, D], bf16).ap()
    aggN = [nc.dram_tensor(f"aggN{L}", [NP, D], bf16).ap() for L in range(2)]
    aggS = [nc.dram_tensor(f"aggS{L}", [NSH, D], bf16).ap() for L in range(2)]

    with tile.TileContext(nc) as tc:
        with tc.tile_pool(name="const", bufs=1) as cp, \
             tc.tile_pool(name="hs", bufs=1) as hp:
            ct = {}
            for name in ("W1ab", "W2m", "b1s", "b2c", "dsqk"):
                t = cp.tile(list(ext[name].shape), ext[name].dtype, tag=name)
                nc.sync.dma_start(t[:], ext[name][:])
                ct[name] = t
            late = {}
            for name in ("gb1s", "gb2s", "gwd1", "gwd2", "wod2", "eidx"):
                lt = cp.tile(list(ext[name].shape), ext[name].dtype, tag=name)
                late[name] = lt
                ct[name] = lt
            # P (5MB) loaded after conv's input DMA is issued -- SP runs its
            # queue in order and P would otherwise delay the conv start; P is
            # first needed by the scatter matmuls ~45us in.
            Pt = cp.tile(list(ext["P"].shape), ext["P"].dtype, tag="P")
            ct["P"] = Pt
            ident = cp.tile([128, 128], bf16, tag="ident")
            make_identity(nc, ident[:])
            dsqd = cp.tile([128, NBC * 128], bf16, tag="dsqd")
            for lb in range(NBC):
                nc.vector.tensor_scalar_mul(
                    dsqd[:, lb * 128:(lb + 1) * 128], ident[:],
                    ct["dsqk"][:, lb:lb + 1])
            y_nb = cp.tile([128, 4 * NBC], f32, tag="ynb")

            hs0 = hp.tile([128, NBC * D], bf16, tag="hs0")
            hs1 = hp.tile([128, NBC * D], bf16, tag="hs1")
            agg_sb0 = hp.tile([128, NBC * D], bf16, tag="asb0")
            agg_sb1 = hp.tile([128, NBC * D], bf16, tag="asb1")
            agg_sb = [agg_sb0, agg_sb1]

            # ---- conv stage: local table0 shard = dsq * relu(conv2(relu(conv1 x)))
            with tc.tile_pool(name="cv", bufs=3) as vp, \
                 tc.tile_pool(name="cvp", bufs=2, space="PSUM") as pp, \
                 tc.tile_pool(name="cvq", bufs=1, space="PSUM") as pq:
                xts = vp.tile([8, NBC * 512], bf16, tag="xts")
                nc.sync.dma_start(xts[:], ext["xt"][:])
                for name in ("gb1s", "gb2s", "gwd1", "gwd2", "wod2", "eidx"):
                    nc.sync.dma_start(late[name][:], ext[name][:])
                nc.sync.dma_start(Pt[:], ext["P"][:])
                for bp in range(NBC // 2):
                    ph1 = pp.tile([128, 1024], f32, tag="ph1", space="PSUM")
                    for hv in range(2):
                        nc.tensor.matmul(
                            ph1[:, hv * 512:(hv + 1) * 512],
                            lhsT=ct["W1ab"][:],
                            rhs=xts[:, bp * 1024 + hv * 512:
                                    bp * 1024 + (hv + 1) * 512],
                            start=True, stop=True)
                    h1 = vp.tile([128, 1024], bf16, tag="h1")
                    nc.vector.tensor_scalar(h1[:], ph1[:],
                                            ct["b1s"][:, 0:1], 0.0,
                                            mybir.AluOpType.add,
                                            mybir.AluOpType.max)
                    ph2 = pq.tile([64, 1024], f32, tag="ph2", space="PSUM")
                    for hv in range(2):
                        nc.tensor.matmul(ph2[:, hv * 512:(hv + 1) * 512],
                                         lhsT=ct["W2m"][:],
                                         rhs=h1[:, hv * 512:(hv + 1) * 512],
                                         start=True, stop=True)
                    h2 = vp.tile([64, 1024], bf16, tag="h2")
                    nc.scalar.activation(h2[:], ph2[:],
                                         mybir.ActivationFunctionType.Relu,
                                         bias=ct["b2c"][:, 0:1])
                    ptp4 = pq.tile([128, 512], bf16, tag="ptp4", space="PSUM")
                    for s in range(8):
                        nc.tensor.transpose(ptp4[:, 64 * s:64 * (s + 1)],
                                            h2[:, s * 128:(s + 1) * 128],
                                            ident[0:64, 0:64])
                    for w in range(2):
                        blk = 2 * bp + w
                        nc.vector.tensor_scalar_mul(
                            hs0[:, blk * D:(blk + 1) * D],
                            ptp4[:, w * D:(w + 1) * D],
                            ct["dsqk"][:, blk:blk + 1])
                nc.sync.dma_start(
                    table0.rearrange("(lb p) f -> p lb f", p=128), hs0[:])

            # ---- GCN layers
            for L in range(2):
                tbl = table0 if L == 0 else table1
                hs_cur = hs0 if L == 0 else hs1
                gwd = ct["gwd1"] if L == 0 else ct["gwd2"]
                gbs = ct["gb1s"] if L == 0 else ct["gb2s"]

                # scatter: partial aggregate over all NP dst rows
                # blocks grouped by 4: one PSUM tile [128, 4, 512] f32 --
                # each block's accumulator bank-aligned (matmul PSUM outputs
                # must start at a bank boundary); one copy + one DMA per group
                GB = 5                   # blocks per group = one shard-half
                with tc.tile_pool(name=f"g{L}", bufs=6) as gp, \
                     tc.tile_pool(name=f"st{L}", bufs=6) as sp, \
                     tc.tile_pool(name=f"sc{L}", bufs=2, space="PSUM") as qp:
                    gleft = {}
                    for specs in slots:
                        for (b, first, last, pbase, cap) in specs:
                            key = (b // 10, 0 if b % 10 < 5 else 1)
                            gleft[key] = gleft.get(key, 0) + 1
                    g = None
                    pbt = {}
                    gcnt = 0
                    mmi = 0
                    for i, specs in enumerate(slots):
                        gi, j = divmod(i, SPG)
                        if j == 0:
                            g = gp.tile([128, SPG, D], bf16, tag="g")
                            nc.gpsimd.dma_gather(
                                g[:], tbl[:],
                                ct["eidx"][:, gi * ICPG:(gi + 1) * ICPG],
                                NIG, NIG, D)
                        for (b, first, last, pbase, cap) in specs:
                            hf, w = (0, b % 10) if b % 10 < 5 else (1, b % 10 - 5)
                            shd = b // 10
                            key = (shd, hf)
                            if first and w == 0:
                                pb4 = qp.tile([128, GB, D], f32, tag="pb4",
                                              space="PSUM")
                                pbt[key] = pb4
                            nc.tensor.matmul(
                                pbt[key][:, w, 0:D],
                                lhsT=ct["P"][:, mmi * 128:(mmi + 1) * 128],
                                rhs=g[:, j, :],
                                start=first, stop=last)
                            mmi += 1
                            gleft[key] -= 1
                            if gleft[key] == 0:
                                stg = sp.tile([128, GB * D], bf16, tag="stg")
                                if (gcnt := gcnt + 1) % 2 == 0:
                                    nc.vector.tensor_copy(stg[:],
                                                          pbt[key][:, :, 0:D])
                                else:
                                    nc.scalar.activation(
                                        stg[:], pbt[key][:, :, 0:D],
                                        mybir.ActivationFunctionType.Copy)
                                ro = shd * 1280 + hf * 640
                                nc.sync.dma_start(
                                    aggN[L][ro:ro + 640, :]
                                    .rearrange("(q p) f -> p q f", p=128),
                                    stg[:])

                nc.gpsimd.collective_compute(
                    "ReduceScatter", mybir.AluOpType.add,
                    replica_groups=[list(range(NCORES))],
                    ins=[aggN[L][:]], outs=[aggS[L][:]])

                # finish: self-loop + dsq + W + bias + relu (local shard),
                # processed in block-pairs to halve instruction count
                with tc.tile_pool(name=f"fv{L}", bufs=7) as fv, \
                     tc.tile_pool(name=f"fp{L}", bufs=2, space="PSUM") as fp, \
                     tc.tile_pool(name=f"fw{L}", bufs=3, space="PSUM") as fw:
                    for pi in range(NBC // 2):
                        nc.sync.dma_start(
                            agg_sb[L][:, pi * 2 * D:(pi + 1) * 2 * D],
                            aggS[L][pi * 256:(pi + 1) * 256, :]
                            .rearrange("(lb p) f -> p lb f", p=128))
                    for pi in range(NBC // 2):
                        b0, b1 = 2 * pi, 2 * pi + 1
                        ta2 = fv.tile([128, 2 * D], bf16, tag="ta2")
                        for w, b in enumerate((b0, b1)):
                            nc.vector.tensor_add(ta2[:, w * D:(w + 1) * D],
                                                 agg_sb[L][:, b * D:(b + 1) * D],
                                                 hs_cur[:, b * D:(b + 1) * D])
                        tp4 = fp.tile([128, 512], f32, tag="tp4", space="PSUM")
                        for c in range(4):   # chunk c = (block w=c//2, pr=c%2)
                            b = 2 * pi + c // 2
                            # transpose + dsq scale in one REGULAR matmul:
                            # out = ta2_sliceT @ diag(dsq_block)
                            nc.tensor.matmul(
                                tp4[:, c * 128:(c + 1) * 128],
                                lhsT=ta2[:, c * 128:(c + 1) * 128],
                                rhs=dsqd[:, b * 128:(b + 1) * 128],
                                start=True, stop=True)
                        tps4 = fv.tile([128, 512], bf16, tag="tps4")
                        if pi % 2 == 0:
                            nc.scalar.activation(
                                tps4[:], tp4[:],
                                mybir.ActivationFunctionType.Copy)
                        else:
                            nc.vector.tensor_copy(tps4[:], tp4[:])
                        wp4 = fw.tile([128, 512], f32, tag="wp4", space="PSUM")
                        nc.tensor.matmul(wp4[:], lhsT=gwd[:], rhs=tps4[:],
                                         start=True, stop=True)
                        h44 = fv.tile([128, 512], bf16, tag="h44")
                        nc.scalar.activation(h44[:], wp4[:],
                                             mybir.ActivationFunctionType.Relu,
                                             bias=gbs[:, 0:1])
                        if L == 0:
                            tb4 = fp.tile([128, 512], bf16, tag="tb4",
                                          space="PSUM")
                            for c in range(4):
                                nc.tensor.transpose(
                                    tb4[:, c * 128:(c + 1) * 128],
                                    h44[:, c * 128:(c + 1) * 128], ident[:])
                            for c in range(4):
                                w, pr = divmod(c, 2)
                                b = 2 * pi + w
                                if c % 2 == 0:
                                    nc.vector.tensor_scalar_mul(
                                        hs1[:, b * D + 128 * pr:
                                            b * D + 128 * (pr + 1)],
                                        tb4[:, c * 128:(c + 1) * 128],
                                        ct["dsqk"][:, b:b + 1])
                                else:
                                    nc.scalar.activation(
                                        hs1[:, b * D + 128 * pr:
                                            b * D + 128 * (pr + 1)],
                                        tb4[:, c * 128:(c + 1) * 128],
                                        mybir.ActivationFunctionType.Copy,
                                        scale=ct["dsqk"][:, b:b + 1])
                            nc.sync.dma_start(
                                table1[pi * 256:(pi + 1) * 256, :]
                                .rearrange("(lb p) f -> p lb f", p=128),
                                hs1[:, pi * 2 * D:(pi + 1) * 2 * D])
                        else:
                            yp4 = fp.tile([128, 8], f32, tag="yp4",
                                          space="PSUM")
                            for c in range(4):
                                nc.tensor.matmul(
                                    yp4[:, c * 2:(c + 1) * 2],
                                    lhsT=h44[:, c * 128:(c + 1) * 128],
                                    rhs=ct["wod2"][:],
                                    start=True, stop=True)
                            nc.vector.tensor_scalar_add(
                                y_nb[:, 8 * pi:8 * pi + 8], yp4[:], bo_f)

            nc.sync.dma_start(y_ext[:], y_nb[:])
    nc.compile()
    return nc


def _run(inputs):
    from concourse.bass_utils import run_bass_kernel_spmd

    in_maps, slots_spl, NGI, bo_f = _host_prep(
        inputs["x"], inputs["edge_index"], inputs["w1"], inputs["b1"],
        inputs["w2"], inputs["b2"], inputs["gw1"], inputs["gb1"],
        inputs["gw2"], inputs["gb2"], inputs["wo"], inputs["bo"])

    key = (hash(tuple(slots_spl[0])), slots_spl[1], NGI)
    if key not in _cache:
        _cache[key] = _build(slots_spl, NGI, bo_f)
    nc = _cache[key]

    res = run_bass_kernel_spmd(nc, in_maps, list(range(8)))
    y = np.zeros((B, N), dtype=np.float32)
    for k in range(NCORES):
        y_nb = res.results[k]["y"]          # [128, 4*NBC]
        for lb in range(NBC):
            lo = k * NSH + lb * 128
            hi = min(lo + 128, N)
            if hi <= lo:
                continue
            for s in range(B):
                y[s, lo:hi] = y_nb[: hi - lo, lb * 4 + s]
    return y


def kernel(**inputs):
    return _run(inputs)


# revision 68
# speedup vs baseline: 1.0009x; 1.0009x over previous
"""GraphWaveNet kernel for Trainium2 (Bass/Tile), 8 NeuronCores.

Design: edge sharding by SOURCE block + per-layer ReduceScatter (no
AllGathers, no replicated compute).

- Only t=11 survives the final 1x1 conv and the GCN doesn't mix time, so
  the conv stack is evaluated at t in {10,11} only and the GCN runs on
  B=4 slices packed as D=256 columns (4 slices x 64 feats).
- GCN identity: with Hs = dsq*h, agg_n = dsq_n * (sum_{e->n} Hs[src_e]
  + Hs[n]), then @W + b + relu.
- Sharding: core k owns node rows [1280k, 1280(k+1)). The conv builds
  only the local shard of the Hs table (bf16, HBM). Edges live on the
  core owning their SRC, so gathers (dma_gather, 1024 indices/instr)
  read only the local table. One-hot P matmuls scatter 128-edge chunks
  into per-dst-block PSUM accumulators (5-block groups, one staging
  copy + one DMA per group), forming a bf16 partial aggregate over all
  10240 rows. A single bf16 ReduceScatter per layer sums the partials
  across cores and returns each core its own 1280 rows; self-loop,
  dsq scaling, weight matmul, bias and relu are then local (block-pair
  batched). Final 1x1 conv + host assembly of y.
- SPMD uniformity: the slot schedule uses S_b = max over cores of
  ceil(edges/128) per dst block; up to three blocks' overflow edges
  share one slot at static partition sub-ranges (the sub-range base is
  host-side data -- the program only sees P columns and chain flags).
  PSUM allows only one open matmul accumulation chain per 2KB bank, so
  chains are emitted in two waves whose open chains never share a bank.
- dma_gather ucode contract (queue 0): flat index j of an instruction
  lands at out[j%128, j//128] and is read from idx tile position
  [16 + j%16, j//16] (int16).
"""

import sys

sys.path.insert(0, "/opt/trn_rl_repo")

import numpy as np
import ml_dtypes

B, T, N, FIN, H, E = 4, 12, 10000, 2, 64, 80000
NCORES = 8
NB80 = 80                 # dst blocks of 128 nodes
NP = NB80 * 128           # padded node count (10240)
NSH = NP // NCORES        # node rows per core (1280)
NBC = NB80 // NCORES      # node blocks per core (10)
D = 4 * H                 # 256 = 4 slices x 64 feats
SPG = 8                   # slots (of 128 edges) per dma_gather (1024 idxs)
NIG = SPG * 128           # indices per gather instruction
ICPG = NIG // 16          # idx tile columns per gather instruction (64)

_cache = {}


def _host_prep(x, edge_index, w1, b1, w2, b2, gw1, gb1, gw2, gb2, wo, bo):
    x = np.asarray(x, np.float32)
    src = np.asarray(edge_index[0]).astype(np.int64)
    dst = np.asarray(edge_index[1]).astype(np.int64)

    deg = np.bincount(dst, minlength=N).astype(np.float64) + 1.0
    dsq = (deg ** -0.5).astype(np.float32)
    dsq_pad = np.ones(NP, dtype=np.float32)
    dsq_pad[:N] = dsq

    # ---- per-core edge partition by src owner, dst-sorted
    owner = src // NSH
    es_k, ed_k, cnt = [], [], np.zeros((NCORES, NB80), np.int64)
    for k in range(NCORES):
        m = owner == k
        es, ed = src[m], dst[m]
        o = np.argsort(ed, kind="stable")
        es, ed = es[o], ed[o]
        es_k.append(es)
        ed_k.append(ed)
        cnt[k] = np.bincount(ed // 128, minlength=NB80)

    S_b = np.maximum(1, (cnt + 127) // 128).max(axis=0)   # slots per block
    ovf_max = np.maximum(0, cnt - 128).max(axis=0)
    assert int(S_b.max()) <= 2 and int(ovf_max.max()) <= 64, \
        "overflow pairing assumes <=192 edges per (core, block)"
    # slots: tuple of mm-specs (block, first, last, pbase). A block's <=128
    # main edges fill its own slot; two blocks' overflow edges (<=64 each)
    # share one slot at partition bases 0 and 64. Groups of 5 blocks (one
    # shard-half) accumulate in one PSUM tile; group's last mm flagged by
    # gdone for the agg copy/DMA.
    # PSUM allows only one OPEN accumulation chain per 2KB bank; regions
    # w={0,1},{2,3},{4} share banks, so emit in two waves whose open chains
    # never share a bank, overflow-pairing within each wave (cross-bank).
    # overflow sub-slot layouts by count: (pbase, cap) pairs; caps must
    # cover max overflow across cores (asserted below)
    OVL = {1: ((0, 128),), 2: ((0, 64), (64, 64)),
           3: ((0, 43), (43, 43), (86, 42))}

    def group_slots(blocks):
        out = []
        for wave in ((blocks[0], blocks[2], blocks[4]),
                     (blocks[1], blocks[3])):
            for b in wave:
                out.append(((b, True, int(S_b[b]) == 1, 0, 128),))
            ovf = [b for b in wave if int(S_b[b]) == 2]
            if ovf:
                lay = OVL[len(ovf)]
                for b, (pb, cap) in zip(ovf, lay):
                    assert int(ovf_max[b]) <= cap, (b, ovf_max[b], cap)
                out.append(tuple((b, False, True, pb, cap)
                                 for b, (pb, cap) in zip(ovf, lay)))
        return out
    slots = []
    for hf in range(2):
        if hf == 1:
            while len(slots) % SPG:
                slots.append(())                # pad to gather-group boundary
            SPL = len(slots)
        for shd in range(NCORES):
            blocks = [10 * shd + 5 * hf + w for w in range(5)]
            slots.extend(group_slots(blocks))
    TOT = len(slots)
    NGI = (TOT + SPG - 1) // SPG

    # eidx layout (dma_gather ucode contract, queue 0): within gather
    # instruction gi, flat index j in [0, NIG) lives at SBUF position
    # [16 + (j % 16), gi * ICPG + j // 16]; edge j lands at out[j%128, j//128].
    NMM = sum(len(s) for s in slots)
    eidx_all = np.zeros((NCORES, 128, NGI * ICPG), np.int16)
    P_all = np.zeros((NCORES, 128, NMM * 128), np.float32)
    for k in range(NCORES):
        es, ed = es_k[k], ed_k[k]
        bounds = np.searchsorted(ed, np.arange(NB80 + 1) * 128)
        mmi = 0
        for si, specs in enumerate(slots):
            for (b, first, last, pbase, cap) in specs:
                e0, e1 = int(bounds[b]), int(bounds[b + 1])
                a = e0 if first else e0 + 128
                bb = min(a + cap, e1)
                ne = max(0, bb - a)
                if ne:
                    gi, c = divmod(si, SPG)
                    jj = c * 128 + pbase + np.arange(ne)   # flat idx in instr
                    eidx_all[k, 16 + (jj % 16), gi * ICPG + jj // 16] = \
                        (es[a:bb] - k * NSH).astype(np.int16)
                    P_all[k, pbase + np.arange(ne),
                          mmi * 128 + (ed[a:bb] - b * 128)] = 1.0
                mmi += 1
    P_all = P_all.astype(ml_dtypes.bfloat16)

    # ---- conv input: per block 8 rows (t,c) for t in {9,10,11} + 2 zero rows,
    # cols = 4 slices x 128 nodes
    xpad = np.zeros((B, 3, FIN, NP), np.float32)
    xpad[:, :, :, :N] = x[:, 9:12, :, :].transpose(0, 1, 3, 2)  # [s, ti, c, n]
    xv = xpad.reshape(B, 6, NCORES, NBC, 128)                   # [s, row, k, blk, p]
    xt_all = np.zeros((NCORES, 8, NBC * 4 * 128), np.float32)
    xt_all[:, :6] = xv.transpose(2, 1, 3, 0, 4).reshape(NCORES, 6, NBC * 4 * 128)
    xt_all = xt_all.astype(ml_dtypes.bfloat16)

    dsqk_all = dsq_pad.reshape(NCORES, NBC, 128).transpose(0, 2, 1).copy()

    # ---- weights
    W1m = np.zeros((6, 64), np.float32)
    for kk in range(3):
        for c in range(FIN):
            W1m[2 * kk + c, :] = w1[:, c, 0, kk]
    W1ab = np.zeros((8, 128), np.float32)
    W1ab[0:6, 0:64] = W1m          # A: t10 (taps t9,t10,t11)
    W1ab[2:8, 64:128] = W1m        # B: t11 (taps t10,t11,t12=pad)
    W1ab = W1ab.astype(ml_dtypes.bfloat16)

    W2m = np.zeros((128, 64), np.float32)
    W2m[:64, :] = w2[:, :, 0, 0].T
    W2m[64:, :] = w2[:, :, 0, 1].T
    W2m = W2m.astype(ml_dtypes.bfloat16)

    b1s = np.concatenate([b1, b1]).reshape(128, 1).astype(np.float32)
    b2c = np.asarray(b2, np.float32).reshape(64, 1)
    gb1s = np.concatenate([gb1, gb1]).reshape(128, 1).astype(np.float32)
    gb2s = np.concatenate([gb2, gb2]).reshape(128, 1).astype(np.float32)
    gwd1 = np.zeros((128, 128), np.float32)
    gwd1[0:64, 0:64] = gw1
    gwd1[64:128, 64:128] = gw1
    gwd1 = gwd1.astype(ml_dtypes.bfloat16)
    gwd2 = np.zeros((128, 128), np.float32)
    gwd2[0:64, 0:64] = gw2
    gwd2[64:128, 64:128] = gw2
    gwd2 = gwd2.astype(ml_dtypes.bfloat16)
    wov = np.asarray(wo, np.float32)[0, :, 0, 0]
    wod2 = np.zeros((128, 2), np.float32)
    wod2[0:64, 0] = wov
    wod2[64:128, 1] = wov
    wod2 = wod2.astype(ml_dtypes.bfloat16)

    shared = {
        "W1ab": W1ab, "W2m": W2m, "b1s": b1s, "b2c": b2c,
        "gb1s": gb1s, "gb2s": gb2s, "gwd1": gwd1, "gwd2": gwd2, "wod2": wod2,
    }
    in_maps = []
    for k in range(NCORES):
        m = dict(shared)
        m["xt"] = xt_all[k]
        m["eidx"] = eidx_all[k]
        m["P"] = P_all[k]
        m["dsqk"] = dsqk_all[k]
        in_maps.append(m)
    return in_maps, (slots, SPL), NGI, float(np.asarray(bo).reshape(-1)[0])


def _build(slots_spl, NGI, bo_f):
    slots, SPL = slots_spl
    from concourse import bass, bacc, tile
    from concourse.masks import make_identity
    import mybir

    f32, bf16, i16 = mybir.dt.float32, mybir.dt.bfloat16, mybir.dt.int16
    f32r = mybir.dt.float32r
    TOT = len(slots)
    NMM = sum(len(s) for s in slots)
    COLS = NGI * ICPG

    nc = bacc.Bacc("TRN2", target_bir_lowering=False, debug=False, num_devices=8,
                   dynamic_dma_scratch_size=65536)

    ext = {}
    for name, shape, dt in [
        ("xt", [8, NBC * 512], bf16), ("W1ab", [8, 128], bf16),
        ("W2m", [128, 64], bf16), ("b1s", [128, 1], f32), ("b2c", [64, 1], f32),
        ("gb1s", [128, 1], f32), ("gb2s", [128, 1], f32),
        ("gwd1", [128, 128], bf16), ("gwd2", [128, 128], bf16),
        ("wod2", [128, 2], bf16), ("dsqk", [128, NBC], f32),
        ("eidx", [128, COLS], i16), ("P", [128, NMM * 128], bf16),
    ]:
        ext[name] = nc.dram_tensor(name, shape, dt, kind="ExternalInput").ap()
    y_ext = nc.dram_tensor("y", [128, 4 * NBC], f32, kind="ExternalOutput").ap()
    table0 = nc.dram_tensor("table0", [NSH, D], bf16).ap()
    table1 = nc.dram_tensor("table1", [NSH, D], bf16).ap()
    aggN = [nc.dram_tensor(f"aggN{L}", [NP, D], bf16).ap() for L in range(2)]
    aggS = [nc.dram_tensor(f"aggS{L}", [NSH, D], bf16).ap() for L in range(2)]

    with tile.TileContext(nc) as tc:
        with tc.tile_pool(name="const", bufs=1) as cp, \
             tc.tile_pool(name="hs", bufs=1) as hp:
            ct = {}
            for name in ("W1ab", "W2m", "b1s", "b2c", "dsqk"):
                t = cp.tile(list(ext[name].shape), ext[name].dtype, tag=name)
                nc.sync.dma_start(t[:], ext[name][:])
                ct[name] = t
            late = {}
            for name in ("gb1s", "gb2s", "gwd1", "gwd2", "wod2", "eidx"):
                lt = cp.tile(list(ext[name].shape), ext[name].dtype, tag=name)
                late[name] = lt
                ct[name] = lt
            # P (5MB) loaded after conv's input DMA is issued -- SP runs its
            # queue in order and P would otherwise delay the conv start; P is
            # first needed by the scatter matmuls ~45us in.
            Pt = cp.tile(list(ext["P"].shape), ext["P"].dtype, tag="P")
            ct["P"] = Pt
            ident = cp.tile([128, 128], bf16, tag="ident")
            make_identity(nc, ident[:])
            dsqd = cp.tile([128, NBC * 128], bf16, tag="dsqd")
            for lb in range(NBC):
                nc.vector.tensor_scalar_mul(
                    dsqd[:, lb * 128:(lb + 1) * 128], ident[:],
                    ct["dsqk"][:, lb:lb + 1])
            y_nb = cp.tile([128, 4 * NBC], f32, tag="ynb")

            hs0 = hp.tile([128, NBC * D], bf16, tag="hs0")
            hs1 = hp.tile([128, NBC * D], bf16, tag="hs1")
            agg_sb0 = hp.tile([128, NBC * D], bf16, tag="asb0")
            agg_sb1 = hp.tile([128, NBC * D], bf16, tag="asb1")
            agg_sb = [agg_sb0, agg_sb1]

            # ---- conv stage: local table0 shard = dsq * relu(conv2(relu(conv1 x)))
            with tc.tile_pool(name="cv", bufs=3) as vp, \
                 tc.tile_pool(name="cvp", bufs=2, space="PSUM") as pp, \
                 tc.tile_pool(name="cvq", bufs=1, space="PSUM") as pq:
                xts = vp.tile([8, NBC * 512], bf16, tag="xts")
                nc.sync.dma_start(xts[:], ext["xt"][:])
                for name in ("gb1s", "gb2s", "gwd1", "gwd2", "wod2", "eidx"):
                    nc.sync.dma_start(late[name][:], ext[name][:])
                nc.sync.dma_start(Pt[:], ext["P"][:])
                for bp in range(NBC // 2):
                    ph1 = pp.tile([128, 1024], f32, tag="ph1", space="PSUM")
                    for hv in range(2):
                        nc.tensor.matmul(
                            ph1[:, hv * 512:(hv + 1) * 512],
                            lhsT=ct["W1ab"][:],
                            rhs=xts[:, bp * 1024 + hv * 512:
                                    bp * 1024 + (hv + 1) * 512],
                            start=True, stop=True)
                    h1 = vp.tile([128, 1024], bf16, tag="h1")
                    nc.vector.tensor_scalar(h1[:], ph1[:],
                                            ct["b1s"][:, 0:1], 0.0,
                                            mybir.AluOpType.add,
                                            mybir.AluOpType.max)
                    ph2 = pq.tile([64, 1024], f32, tag="ph2", space="PSUM")
                    for hv in range(2):
                        nc.tensor.matmul(ph2[:, hv * 512:(hv + 1) * 512],
                                         lhsT=ct["W2m"][:],
                                         rhs=h1[:, hv * 512:(hv + 1) * 512],
                                         start=True, stop=True)
                    h2 = vp.tile([64, 1024], bf16, tag="h2")
                    nc.scalar.activation(h2[:], ph2[:],
                                         mybir.ActivationFunctionType.Relu,
                                         bias=ct["b2c"][:, 0:1])
                    ptp4 = pq.tile([128, 512], bf16, tag="ptp4", space="PSUM")
                    for s in range(8):
                        nc.tensor.transpose(ptp4[:, 64 * s:64 * (s + 1)],
                                            h2[:, s * 128:(s + 1) * 128],
                                            ident[0:64, 0:64])
                    for w in range(2):
                        blk = 2 * bp + w
                        nc.vector.tensor_scalar_mul(
                            hs0[:, blk * D:(blk + 1) * D],
                            ptp4[:, w * D:(w + 1) * D],
                            ct["dsqk"][:, blk:blk + 1])
                nc.sync.dma_start(
                    table0.rearrange("(lb p) f -> p lb f", p=128), hs0[:])

            # ---- GCN layers
            for L in range(2):
                tbl = table0 if L == 0 else table1
                hs_cur = hs0 if L == 0 else hs1
                gwd = ct["gwd1"] if L == 0 else ct["gwd2"]
                gbs = ct["gb1s"] if L == 0 else ct["gb2s"]

                # scatter: partial aggregate over all NP dst rows
                # blocks grouped by 4: one PSUM tile [128, 4, 512] f32 --
                # each block's accumulator bank-aligned (matmul PSUM outputs
                # must start at a bank boundary); one copy + one DMA per group
                GB = 5                   # blocks per group = one shard-half
                with tc.tile_pool(name=f"g{L}", bufs=6) as gp, \
                     tc.tile_pool(name=f"st{L}", bufs=6) as sp, \
                     tc.tile_pool(name=f"sc{L}", bufs=2, space="PSUM") as qp:
                    gleft = {}
                    for specs in slots:
                        for (b, first, last, pbase, cap) in specs:
                            key = (b // 10, 0 if b % 10 < 5 else 1)
                            gleft[key] = gleft.get(key, 0) + 1
                    g = None
                    pbt = {}
                    gcnt = 0
                    mmi = 0
                    for i, specs in enumerate(slots):
                        gi, j = divmod(i, SPG)
                        if j == 0:
                            g = gp.tile([128, SPG, D], bf16, tag="g")
                            nc.gpsimd.dma_gather(
                                g[:], tbl[:],
                                ct["eidx"][:, gi * ICPG:(gi + 1) * ICPG],
                                NIG, NIG, D)
                        for (b, first, last, pbase, cap) in specs:
                            hf, w = (0, b % 10) if b % 10 < 5 else (1, b % 10 - 5)
                            shd = b // 10
                            key = (shd, hf)
                            if first and w == 0:
                                pb4 = qp.tile([128, GB, D], f32, tag="pb4",
                                              space="PSUM")
                                pbt[key] = pb4
                            nc.tensor.matmul(
                                pbt[key][:, w, 0:D],
                                lhsT=ct["P"][:, mmi * 128:(mmi + 1) * 128],
                                rhs=g[:, j, :],
                                start=first, stop=last)
                            mmi += 1
                            gleft[key] -= 1
                            if gleft[key] == 0:
                                stg = sp.tile([128, GB * D], bf16, tag="stg")
                                if (gcnt := gcnt + 1) % 2 == 0:
                                    nc.vector.tensor_copy(stg[:],
                                                          pbt[key][:, :, 0:D])
                                else:
                                    nc.scalar.activation(
                                        stg[:], pbt[key][:, :, 0:D],
                                        mybir.ActivationFunctionType.Copy)
                                ro = shd * 1280 + hf * 640
                                nc.sync.dma_start(
                                    aggN[L][ro:ro + 640, :]
                                    .rearrange("(q p) f -> p q f", p=128),
                                    stg[:])

                nc.gpsimd.collective_compute(
                    "ReduceScatter", mybir.AluOpType.add,
                    replica_groups=[list(range(NCORES))],
                    ins=[aggN[L][:]], outs=[aggS[L][:]])

                # finish: self-loop + dsq + W + bias + relu (local shard),
                # processed in block-pairs to halve instruction count
                with tc.tile_pool(name=f"fv{L}", bufs=7) as fv, \
                     tc.tile_pool(name=f"fp{L}", bufs=2, space="PSUM") as fp, \
                     tc.tile_pool(name=f"fw{L}", bufs=3, space="PSUM") as fw:
                    for pi in range(NBC // 2):
                        nc.sync.dma_start(
                            agg_sb[L][:, pi * 2 * D:(pi + 1) * 2 * D],
                            aggS[L][pi * 256:(pi + 1) * 256, :]
                            .rearrange("(lb p) f -> p lb f", p=128))
                    for pi in range(NBC // 2):
                        b0, b1 = 2 * pi, 2 * pi + 1
                        ta2 = fv.tile([128, 2 * D], bf16, tag="ta2")
                        for w, b in enumerate((b0, b1)):
                            nc.vector.tensor_add(ta2[:, w * D:(w + 1) * D],
                                                 agg_sb[L][:, b * D:(b + 1) * D],
                                                 hs_cur[:, b * D:(b + 1) * D])
                        tp4 = fp.tile([128, 512], f32, tag="tp4", space="PSUM")
                        for c in range(4):   # chunk c = (block w=c//2, pr=c%2)
                            b = 2 * pi + c // 2
                            # transpose + dsq scale in one REGULAR matmul:
                            # out = ta2_sliceT @ diag(dsq_block)
                            nc.tensor.matmul(
                                tp4[:, c * 128:(c + 1) * 128],
                                lhsT=ta2[:, c * 128:(c + 1) * 128],
                                rhs=dsqd[:, b * 128:(b + 1) * 128],
                                start=True, stop=True)
                        tps4 = fv.tile([128, 512], bf16, tag="tps4")
                        nc.vector.tensor_copy(tps4[:], tp4[:])
                        wp4 = fw.tile([128, 512], f32, tag="wp4", space="PSUM")
                        nc.tensor.matmul(wp4[:], lhsT=gwd[:], rhs=tps4[:],
                                         start=True, stop=True)
                        h44 = fv.tile([128, 512], bf16, tag="h44")
                        nc.scalar.activation(h44[:], wp4[:],
                                             mybir.ActivationFunctionType.Relu,
                                             bias=gbs[:, 0:1])
                        if L == 0:
                            tb4 = fp.tile([128, 512], bf16, tag="tb4",
                                          space="PSUM")
                            for c in range(4):
                                nc.tensor.transpose(
                                    tb4[:, c * 128:(c + 1) * 128],
                                    h44[:, c * 128:(c + 1) * 128], ident[:])
                            for c in range(4):
                                w, pr = divmod(c, 2)
                                b = 2 * pi + w
                                if c % 2 == 0:
                                    nc.vector.tensor_scalar_mul(
                                        hs1[:, b * D + 128 * pr:
                                            b * D + 128 * (pr + 1)],
                                        tb4[:, c * 128:(c + 1) * 128],
                                        ct["dsqk"][:, b:b + 1])
                                else:
                                    nc.scalar.activation(
                                        hs1[:, b * D + 128 * pr:
                                            b * D + 128 * (pr + 1)],
                                        tb4[:, c * 128:(c + 1) * 128],
                                        mybir.ActivationFunctionType.Copy,
                                        scale=ct["dsqk"][:, b:b + 1])
                            nc.sync.dma_start(
                                table1[pi * 256:(pi + 1) * 256, :]
                                .rearrange("(lb p) f -> p lb f", p=128),
                                hs1[:, pi * 2 * D:(pi + 1) * 2 * D])
                        else:
                            yp4 = fp.tile([128, 8], f32, tag="yp4",
                                          space="PSUM")
                            for c in range(4):
                                nc.tensor.matmul(
                                    yp4[:, c * 2:(c + 1) * 2],
                                    lhsT=h44[:, c * 128:(c + 1) * 128],
                                    rhs=ct["wod2"][:],
                                    start=True, stop=True)
                            nc.vector.tensor_scalar_add(
                                y_nb[:, 8 * pi:8 * pi + 8], yp4[:], bo_f)

            nc.sync.dma_start(y_ext[:], y_nb[:])
    nc.compile()
    return nc


def _run(inputs):
    from concourse.bass_utils import run_bass_kernel_spmd

    in_maps, slots_spl, NGI, bo_f = _host_prep(
        inputs["x"], inputs["edge_index"], inputs["w1"], inputs["b1"],
        inputs["w2"], inputs["b2"], inputs["gw1"], inputs["gb1"],
        inputs["gw2"], inputs["gb2"], inputs["wo"], inputs["bo"])

    key = (hash(tuple(slots_spl[0])), slots_spl[1], NGI)
    if key not in _cache:
        _cache[key] = _build(slots_spl, NGI, bo_f)
    nc = _cache[key]

    res = run_bass_kernel_spmd(nc, in_maps, list(range(8)))
    y = np.zeros((B, N), dtype=np.float32)
    for k in range(NCORES):
        y_nb = res.results[k]["y"]          # [128, 4*NBC]
        for lb in range(NBC):
            lo = k * NSH + lb * 128
            hi = min(lo + 128, N)
            if hi <= lo:
                continue
            for s in range(B):
                y[s, lo:hi] = y_nb[: hi - lo, lb * 4 + s]
    return y


def kernel(**inputs):
    return _run(inputs)
